# revision 9
# baseline (speedup 1.0000x reference)
"""Bass/Tile TRN2 kernel for nn_DifferentialWordSegmentation.

kernel(**inputs) takes the FULL unsharded inputs (numpy), shards batch B=32
across 8 NeuronCores (4 rows each, pure data parallel), runs one SPMD Bass
kernel, and returns the full (32, 1024, 512) float32 output.

Self-contained: shapes/sharding hardcoded, no sibling imports.
"""
import os
import numpy as np

import concourse.bacc as bacc
import concourse.mybir as mybir
import concourse.tile as tile
from concourse.bass_utils import run_bass_kernel_spmd

F32 = mybir.dt.float32
F32R = mybir.dt.float32r
BF16 = mybir.dt.bfloat16
AF = mybir.ActivationFunctionType
OP = mybir.AluOpType

B, N, H = 32, 1024, 512
NCORES = 8
RPC = B // NCORES          # rows per core = 4
NT = N // 128              # 8 i-tiles
HT = H // 128              # 4 h-tiles
THR = 0.05

DEBUG = bool(int(os.environ.get("KERNEL_DEBUG", "0")))

_cached = {}


def _build_module():
    nc = bacc.Bacc(trn_type="TRN2", target_bir_lowering=False, debug=False)

    x_d = nc.dram_tensor("x", [RPC, N, H], F32, kind="ExternalInput").ap()
    mask_d = nc.dram_tensor("mask", [RPC, N], F32, kind="ExternalInput").ap()
    W1_d = nc.dram_tensor("W1", [2 * H, H], F32, kind="ExternalInput").ap()
    b1_d = nc.dram_tensor("b1", [H], F32, kind="ExternalInput").ap()
    W2_d = nc.dram_tensor("W2", [H, 1], F32, kind="ExternalInput").ap()
    We1_d = nc.dram_tensor("We1", [H, H], F32, kind="ExternalInput").ap()
    be1_d = nc.dram_tensor("be1", [H], F32, kind="ExternalInput").ap()
    We2_d = nc.dram_tensor("We2", [H, H], F32, kind="ExternalInput").ap()
    be2_d = nc.dram_tensor("be2", [H], F32, kind="ExternalInput").ap()
    iota_d = nc.dram_tensor("iota1024", [1, N], F32, kind="ExternalInput").ap()
    i128_d = nc.dram_tensor("iota128", [1, 128], F32, kind="ExternalInput").ap()
    idx_d = nc.dram_tensor("idx128", [128, 1], F32, kind="ExternalInput").ap()
    out_d = nc.dram_tensor("out", [RPC, N, H], F32, kind="ExternalOutput").ap()
    dumps = {}
    if DEBUG:
        for nm, shp in (("S_dump", [RPC, N]), ("D_dump", [RPC, N]),
                        ("P_dump", [RPC, N]), ("b_dump", [RPC, N]),
                        ("c_dump", [RPC, N]), ("cnt_dump", [RPC, N]),
                        ("wr_dump", [RPC, H, N])):
            dumps[nm] = nc.dram_tensor(nm, shp, F32, kind="ExternalOutput").ap()

    with tile.TileContext(nc) as tc:
        _emit(nc, tc, x_d, mask_d, W1_d, b1_d, W2_d, We1_d, be1_d, We2_d,
              be2_d, iota_d, i128_d, idx_d, out_d, dumps)
    nc.compile()
    return nc


def _emit(nc, tc, x_d, mask_d, W1_d, b1_d, W2_d, We1_d, be1_d, We2_d, be2_d,
          iota_d, i128_d, idx_d, out_d, dumps):
    from contextlib import ExitStack
    ctx = ExitStack()
    pool = lambda name, bufs, **kw: ctx.enter_context(
        tc.tile_pool(name=name, bufs=bufs, **kw))

    const = pool("const", 1)
    wpool = pool("weights", 1)
    xn_p = pool("xn", 1)       # (128, 512) f32 tiles, tags xn0..7
    xr_p = pool("xr", 1)       # (128, 512) f32r, tags xr0..7
    big_a = pool("biga", 1)    # (128, N+8) XNT / (128, N) wr, tags bg0..3
    big_b = pool("bigb", 1)    # (128, N) relu1 / r1m, tags bb0..3
    sg_p = pool("sg", 1)       # (128, 512) f32r, tags sg0..7
    scr = pool("scratch", 2)
    scr1 = pool("scratch1", 1)
    tiny = pool("tiny", 2)
    wide1 = pool("wide1", 1)
    cpool = pool("phasec", 1)
    outp = pool("outstage", 3)
    psA = pool("psA", 2, space="PSUM")
    psB = pool("psB", 4, space="PSUM")
    psS = pool("psS", 1, space="PSUM")

    # ---------------- constants / weights ----------------
    iota_bc = const.tile([128, N], F32, name="iota_bc")     # rows of 1..1024
    nc.sync.dma_start(iota_bc[:], iota_d.to_broadcast((128, N)))
    i128_bc = const.tile([128, 128], F32, name="i128_bc")   # rows of 0..127
    nc.sync.dma_start(i128_bc[:], i128_d.to_broadcast((128, 128)))
    idxcol = const.tile([128, 1], F32, name="idxcol")       # 0..127
    nc.sync.dma_start(idxcol[:], idx_d)
    be2_bc = const.tile([128, H], F32, name="be2_bc")
    nc.sync.dma_start(be2_bc[:],
                      be2_d.rearrange("(o h) -> o h", o=1).to_broadcast((128, H)))
    zeros_bc = const.tile([128, N], F32, name="zeros_bc")
    nc.vector.memset(zeros_bc[:], 0.0)
    ones_r = const.tile([128, 1], F32R, name="ones_r")
    nc.vector.tensor_scalar(ones_r[:], idxcol[:], -1.0, None, op0=OP.is_gt)
    ident4 = const.tile([4, 4], F32, name="ident4")
    nc.vector.tensor_scalar(ident4[:], i128_bc[0:4, 0:4], idxcol[0:4, :], None,
                            op0=OP.is_equal)
    ident128 = const.tile([128, 128], F32, name="ident128")
    nc.vector.tensor_scalar(ident128[:], i128_bc[:, 0:128], idxcol[:], None,
                            op0=OP.is_equal)

    W1a = [wpool.tile([128, H], F32, name=f"w1a_{k}") for k in range(HT)]
    W1b = [wpool.tile([128, H], F32, name=f"w1b_{k}") for k in range(HT)]
    We1r = [wpool.tile([128, H], F32R, name=f"we1r_{k}") for k in range(HT)]
    We2r = [wpool.tile([128, H], F32R, name=f"we2r_{k}") for k in range(HT)]
    for k in range(HT):
        nc.sync.dma_start(W1a[k][:], W1_d[k * 128:(k + 1) * 128, :])
        nc.sync.dma_start(W1b[k][:], W1_d[H + k * 128:H + (k + 1) * 128, :])
        wtmp1 = scr.tile([128, H], F32, name="wtmp1", tag="wtmp")
        nc.sync.dma_start(wtmp1[:], We1_d[k * 128:(k + 1) * 128, :])
        nc.gpsimd.tensor_copy(We1r[k][:], wtmp1[:])
        wtmp2 = scr.tile([128, H], F32, name="wtmp2", tag="wtmp")
        nc.sync.dma_start(wtmp2[:], We2_d[k * 128:(k + 1) * 128, :])
        nc.gpsimd.tensor_copy(We2r[k][:], wtmp2[:])
    w2c = wpool.tile([128, HT], F32, name="w2c")
    w2_v = W2_d.rearrange("(k p) o -> k p o", p=128)
    b1c = wpool.tile([128, HT], F32, name="b1c")
    b1_v = b1_d.rearrange("(k p) -> k p", p=128)
    be1c = wpool.tile([128, HT], F32, name="be1c")
    be1_v = be1_d.rearrange("(k p) -> k p", p=128)
    for k in range(HT):
        nc.sync.dma_start(w2c[:, k:k + 1], w2_v[k])
        nc.sync.dma_start(b1c[:, k:k + 1], b1_v[k].unsqueeze(1))
        nc.sync.dma_start(be1c[:, k:k + 1], be1_v[k].unsqueeze(1))
    mask4 = const.tile([RPC, N], F32, name="mask4")
    nc.sync.dma_start(mask4[:], mask_d)

    Srow = cpool.tile([RPC, N], F32, name="Srow")

    # ------------- stage 1 per row: load, norms, transpose, G, S -------------
    for r in range(RPC):
        xnat = [xn_p.tile([128, H], F32, name=f"xn_{r}_{t}", tag=f"xn{t}")
                for t in range(NT)]
        ssq = tiny.tile([128, NT], F32, name=f"ssq_{r}", tag="ssq")
        for t in range(NT):
            nc.sync.dma_start(xnat[t][:], x_d[r, t * 128:(t + 1) * 128, :])
            sqs = scr.tile([128, H], F32, name="sqs", tag="sqs")
            nc.scalar.activation(sqs[:], xnat[t][:], AF.Square,
                                 accum_out=ssq[:, t:t + 1])
        rno = tiny.tile([128, NT], F32, name=f"rno_{r}", tag="rno")
        nc.scalar.activation(rno[:], ssq[:], AF.Sqrt)
        rn = tiny.tile([128, NT], F32, name=f"rn_{r}", tag="rn")
        nc.vector.reciprocal(rn[:], rno[:])

        XNT = [big_a.tile([128, N + 8], F32, name=f"xnt_{r}_{k}", tag=f"bg{k}")
               for k in range(HT)]
        for k in range(HT):
            nc.vector.memset(XNT[k][:, N:], 0.0)
        for t in range(NT):
            xsc = scr.tile([128, H], F32, name="xsc", tag="xsc")
            nc.scalar.mul(xsc[:], xnat[t][:], rn[:, t:t + 1])
            for k in range(HT):
                pst = psA.tile([128, 128], F32, name="pst", tag="pst")
                nc.tensor.transpose(pst[:], xsc[:, k * 128:(k + 1) * 128],
                                    ident128[:])
                nc.vector.tensor_copy(XNT[k][:, t * 128:(t + 1) * 128], pst[:])

        relu1 = [big_b.tile([128, N], F32, name=f"r1_{r}_{j}", tag=f"bb{j}")
                 for j in range(HT)]
        for c in range(2):
            for j in range(HT):
                psg = psB.tile([128, 512], F32, name="psg", tag="mm")
                for k in range(HT):
                    nc.tensor.matmul(psg[:], W1a[k][:, j * 128:(j + 1) * 128],
                                     XNT[k][:, c * 512:c * 512 + 512],
                                     start=(k == 0), stop=False)
                for k in range(HT):
                    nc.tensor.matmul(psg[:], W1b[k][:, j * 128:(j + 1) * 128],
                                     XNT[k][:, c * 512 + 1:c * 512 + 513],
                                     start=False, stop=(k == HT - 1))
                nc.scalar.activation(relu1[j][:, c * 512:(c + 1) * 512], psg[:],
                                     AF.Relu, bias=b1c[:, j:j + 1])
            pss = psS.tile([1, 512], F32, name="pss", tag="pss")
            for k in range(HT):
                nc.tensor.matmul(pss[:], w2c[:, k:k + 1],
                                 relu1[k][:, c * 512:(c + 1) * 512],
                                 start=(k == 0), stop=(k == HT - 1))
            stmp = tiny.tile([1, 512], F32, name="stmp", tag="stmp")
            nc.vector.tensor_copy(stmp[:], pss[:])
            nc.sync.dma_start(Srow[r:r + 1, c * 512:(c + 1) * 512], stmp[:])

    # ---------------- stage 2: phase C on (RPC, N) tiles ----------------
    NV = N - 1  # 1023 valid S columns
    Smax = cpool.tile([RPC, 1], F32, name="Smax")
    Smin = cpool.tile([RPC, 1], F32, name="Smin")
    nc.vector.tensor_reduce(Smax[:], Srow[:, 0:NV], axis=mybir.AxisListType.X,
                            op=OP.max)
    nc.vector.tensor_reduce(Smin[:], Srow[:, 0:NV], axis=mybir.AxisListType.X,
                            op=OP.min)
    nrng = cpool.tile([RPC, 1], F32, name="nrng")
    nc.vector.tensor_tensor(nrng[:], Smin[:], Smax[:], op=OP.subtract)
    nrinv = cpool.tile([RPC, 1], F32, name="nrinv")
    nc.vector.reciprocal(nrinv[:], nrng[:])
    if dumps:
        nc.sync.dma_start(dumps["S_dump"], Srow[:])
    D = Srow
    nc.vector.tensor_scalar(D[:], Srow[:], Smax[:], nrinv[:],
                            op0=OP.subtract, op1=OP.mult)
    if dumps:
        nc.sync.dma_start(dumps["D_dump"], D[:])

    fo = cpool.tile([RPC, N], F32, name="fo")
    so = cpool.tile([RPC, N], F32, name="so")
    nc.vector.memset(fo[:], 0.0)
    nc.vector.memset(so[:], 0.0)
    ta = cpool.tile([RPC, N], F32, name="ta")
    tb = cpool.tile([RPC, N], F32, name="tb")
    L = 1020   # fo interior i = 1..1020 (1021/1022 use the overwrite formula)
    nc.vector.tensor_tensor(ta[:, 0:L], D[:, 1:1 + L], D[:, 0:L], op=OP.subtract)
    nc.vector.tensor_scalar(ta[:, 0:L], ta[:, 0:L], 0.0, None, op0=OP.max)
    nc.vector.tensor_tensor(tb[:, 0:L], D[:, 1:1 + L], D[:, 2:2 + L], op=OP.subtract)
    nc.vector.tensor_scalar(tb[:, 0:L], tb[:, 0:L], 0.0, None, op0=OP.max)
    nc.vector.tensor_tensor(fo[:, 1:1 + L], tb[:, 0:L], ta[:, 0:L], op=OP.min)
    nc.vector.tensor_tensor(ta[:, 0:1], D[:, 0:1], D[:, 1:2], op=OP.subtract)
    nc.vector.tensor_scalar(fo[:, 0:1], ta[:, 0:1], 0.0, None, op0=OP.max)
    nc.vector.tensor_tensor(ta[:, 0:2], D[:, 1021:1023], D[:, 1019:1021],
                            op=OP.subtract)
    nc.vector.tensor_scalar(fo[:, 1021:1023], ta[:, 0:2], 0.0, None, op0=OP.max)
    L2 = 1019  # so interior i = 2..1020
    nc.vector.tensor_tensor(ta[:, 0:L2], D[:, 2:2 + L2], D[:, 0:L2], op=OP.subtract)
    nc.vector.tensor_scalar(ta[:, 0:L2], ta[:, 0:L2], 0.0, None, op0=OP.max)
    nc.vector.tensor_tensor(tb[:, 0:L2], D[:, 2:2 + L2], D[:, 4:4 + L2],
                            op=OP.subtract)
    nc.vector.tensor_scalar(tb[:, 0:L2], tb[:, 0:L2], 0.0, None, op0=OP.max)
    nc.vector.tensor_tensor(so[:, 2:2 + L2], tb[:, 0:L2], ta[:, 0:L2], op=OP.min)
    nc.vector.tensor_tensor(ta[:, 0:2], D[:, 0:2], D[:, 2:4], op=OP.subtract)
    nc.vector.tensor_scalar(so[:, 0:2], ta[:, 0:2], 0.0, None, op0=OP.max)

    P = cpool.tile([RPC, N], F32, name="P")
    nc.vector.memset(P[:], 0.0)
    nc.vector.tensor_tensor(ta[:, 0:NV], fo[:, 0:NV], so[:, 0:NV], op=OP.max)
    nc.vector.tensor_scalar(ta[:, 0:NV], ta[:, 0:NV], THR, 0.0,
                            op0=OP.subtract, op1=OP.max)
    nc.vector.tensor_tensor(P[:, 0:NV], ta[:, 0:NV], fo[:, 0:NV], op=OP.min)
    # P = relu(P + (mask - 1)) with reference op order
    nc.vector.tensor_scalar(so[:], mask4[:], 1.0, None, op0=OP.subtract)
    nc.vector.tensor_tensor(P[:], P[:], so[:], op=OP.add)
    nc.vector.tensor_scalar(P[:], P[:], 0.0, None, op0=OP.max)
    if dumps:
        nc.sync.dma_start(dumps["P_dump"], P[:])
    # straight-through boundaries: b = bs + (bh - bs)
    nc.scalar.activation(ta[:], P[:], AF.Tanh, scale=10.0)
    nc.scalar.activation(tb[:], P[:], AF.Tanh, scale=100000.0)
    nc.vector.tensor_tensor(tb[:], tb[:], ta[:], op=OP.subtract)
    nc.vector.tensor_tensor(ta[:], ta[:], tb[:], op=OP.add)
    if dumps:
        nc.sync.dma_start(dumps["b_dump"], ta[:])
    # cumsum along i, then +1 where first element == 0
    cc = cpool.tile([RPC, N], F32, name="cc")
    nc.vector.tensor_tensor_scan(cc[:], ta[:], zeros_bc[0:RPC, :], 0.0,
                                 op0=OP.add, op1=OP.add)
    ind0 = cpool.tile([RPC, 1], F32, name="ind0")
    nc.vector.tensor_scalar(ind0[:], cc[:, 0:1], 0.0, None, op0=OP.is_equal)
    nc.vector.tensor_scalar(cc[:], cc[:], ind0[:], None, op0=OP.add)
    if dumps:
        nc.sync.dma_start(dumps["c_dump"], cc[:])
    # transpose c to (i, r): ct[:, t*RPC + r]
    ct = cpool.tile([128, NT * RPC], F32, name="ct")
    for t in range(NT):
        psc = psA.tile([128, RPC], F32, name="psc", tag="pst")
        nc.tensor.transpose(psc[:], cc[:, t * 128:(t + 1) * 128], ident4[:])
        nc.vector.tensor_copy(ct[:, t * RPC:(t + 1) * RPC], psc[:])

    # ------------- stage 3/4 per row: Wseg, pooling, MLP, store -------------
    for r in range(RPC):
        xr = [xr_p.tile([128, H], F32R, name=f"xr_{r}_{t}", tag=f"xr{t}")
              for t in range(NT)]
        for t in range(NT):
            xs = xn_p.tile([128, H], F32, name=f"xs_{r}_{t}", tag=f"xn{t}")
            nc.sync.dma_start(xs[:], x_d[r, t * 128:(t + 1) * 128, :])
            nc.gpsimd.tensor_copy(xr[t][:], xs[:])
        wr = [big_a.tile([128, N], F32R, name=f"wr_{r}_{k}", tag=f"bg{k}")
              for k in range(HT)]
        cntrow = wide1.tile([1, N], F32, name=f"cnt_{r}", tag="cnt")
        for c in range(2):
            sgs = []
            for t in range(NT):
                ut = scr.tile([128, 512], F32, name="ut", tag="ut")
                nc.vector.tensor_scalar(ut[:], iota_bc[:, c * 512:(c + 1) * 512],
                                        ct[:, t * RPC + r:t * RPC + r + 1], None,
                                        op0=OP.subtract)
                nc.scalar.activation(ut[:], ut[:], AF.Abs)
                sg = sg_p.tile([128, 512], F32R, name=f"sg_{t}", tag=f"sg{t}")
                # XLA f32 tanh saturates to 1.0 at |x| >= 7.90531110763549805,
                # which is what decides membership in the reference; the window
                # value itself cancels in the column normalization.
                nc.vector.tensor_scalar(sg[:], ut[:], 7.90531110763549805e-5,
                                        None, op0=OP.is_lt)
                sgs.append(sg)
            for hh in range(HT):
                psp = psB.tile([128, 512], F32, name="psp", tag="mm")
                for t in range(NT):
                    nc.tensor.matmul(psp[:], xr[t][:, hh * 128:(hh + 1) * 128],
                                     sgs[t][:], start=(t == 0), stop=(t == NT - 1))
                nc.vector.tensor_copy(wr[hh][:, c * 512:(c + 1) * 512], psp[:])
            pscnt = psS.tile([1, 512], F32, name="pscnt", tag="pss")
            for t in range(NT):
                nc.tensor.matmul(pscnt[:], ones_r[:], sgs[t][:],
                                 start=(t == 0), stop=(t == NT - 1))
            nc.vector.tensor_scalar(cntrow[0:1, c * 512:(c + 1) * 512], pscnt[:],
                                    1e-30, None, op0=OP.max)
        factor = wide1.tile([1, N], F32, name=f"fac_{r}", tag="fac")
        nc.vector.reciprocal(factor[:], cntrow[:])
        fbc = scr1.tile([128, N], F32, name="fbc", tag="fbc")
        nc.gpsimd.partition_broadcast(fbc[:], factor[:])
        for hh in range(HT):
            nc.vector.tensor_tensor(wr[hh][:], wr[hh][:], fbc[:], op=OP.mult)
        if dumps:
            nc.sync.dma_start(dumps["cnt_dump"][r:r + 1, :], cntrow[:])
            for hh in range(HT):
                for c in range(2):
                    wrtmp = scr.tile([128, 512], F32, name="wrtmp", tag="ut")
                    nc.vector.tensor_copy(wrtmp[:], wr[hh][:, c * 512:(c + 1) * 512])
                    nc.sync.dma_start(
                        dumps["wr_dump"][r, hh * 128:(hh + 1) * 128,
                                         c * 512:(c + 1) * 512], wrtmp[:])
        # MLP layer 1: r1m (j, m) = relu(We1.T @ wr + be1)
        r1m = [big_b.tile([128, N], F32R, name=f"r1m_{r}_{j}", tag=f"bb{j}")
               for j in range(HT)]
        for c in range(2):
            for j in range(HT):
                psm = psB.tile([128, 512], F32, name="psm", tag="mm")
                for k in range(HT):
                    nc.tensor.matmul(psm[:], We1r[k][:, j * 128:(j + 1) * 128],
                                     wr[k][:, c * 512:(c + 1) * 512],
                                     start=(k == 0), stop=(k == HT - 1))
                nc.scalar.activation(r1m[j][:, c * 512:(c + 1) * 512], psm[:],
                                     AF.Relu, bias=be1c[:, j:j + 1])
        # MLP layer 2 (natural out): out(m, h) = r1m.T @ We2 + be2
        for mt in range(NT):
            pso = psB.tile([128, 512], F32, name="pso", tag="mm")
            for j in range(HT):
                nc.tensor.matmul(pso[:], r1m[j][:, mt * 128:(mt + 1) * 128],
                                 We2r[j][:], start=(j == 0), stop=(j == HT - 1))
            ot = outp.tile([128, H], F32, name="ot", tag="ot")
            nc.vector.tensor_tensor(ot[:], pso[:], be2_bc[:], op=OP.add)
            nc.sync.dma_start(out_d[r, mt * 128:(mt + 1) * 128, :], ot[:])
    ctx.close()


def _get_module():
    if "nc" not in _cached:
        _cached["nc"] = _build_module()
    return _cached["nc"]


def _make_in_maps(inputs):
    x = np.ascontiguousarray(np.asarray(inputs["segment_rep"], dtype=np.float32))
    mask = np.ascontiguousarray(np.asarray(inputs["phn_mask"], dtype=np.float32))
    shared = {k: np.ascontiguousarray(np.asarray(inputs[k], np.float32))
              for k in ("W1", "b1", "W2", "We1", "be1", "We2", "be2")}
    shared["iota1024"] = np.arange(1, N + 1, dtype=np.float32).reshape(1, N)
    shared["iota128"] = np.arange(128, dtype=np.float32).reshape(1, 128)
    shared["idx128"] = np.arange(128, dtype=np.float32).reshape(128, 1)
    in_maps = []
    for core in range(NCORES):
        m = dict(shared)
        m["x"] = x[core * RPC:(core + 1) * RPC]
        m["mask"] = mask[core * RPC:(core + 1) * RPC]
        in_maps.append(m)
    return in_maps


def run_raw(inputs):
    """Run the SPMD kernel; returns list of per-core result dicts."""
    nc = _get_module()
    in_maps = _make_in_maps(inputs)
    res = run_bass_kernel_spmd(nc, in_maps, list(range(NCORES)))
    return res.results


def kernel(**inputs) -> np.ndarray:
    results = run_raw(inputs)
    out = np.concatenate([r["out"] for r in results], axis=0)
    return out.astype(np.float32)
